# revision 23
# baseline (speedup 1.0000x reference)
"""Multi-head factorized dense attention on 8 TRN2 NeuronCores.

Reference computation (per batch b):
    V = x @ Wv                      (4096, 256)
    l = x @ Wl, r = x @ Wr          (4096, 64) each
    attn[n, p*64+q] = l[n,p]*r[n,q] (4096, 4096)
    score = softmax(attn, -1)
    o = score @ V                   (shared across heads == plain matmul)
    out = o @ Wo

Sharding: 8 cores = 2 batches x 4 query-row chunks of 1024 rows.

Host precomputes the small dense projections (l, r, V with an appended
ones-column, exact per-row score max via the outer-product corner trick)
and applies Wo to the gathered device output.  The device does only the
O(S^2) part: outer product -> exp -> transpose -> score @ [V | 1].

E is stored as fp8e4m3 scaled by 240 (folded into the exp bias; the
scale cancels in the softmax normalization).  The fp8 E is transposed
through the XBAR viewed as fp16 pairs — half the transpose cost — which
lands m=256c+2p+u at partition p of pair-chunk c.  Each pair-chunk is
contracted with two stride-2 fp8 weight matmuls against fp16 V rows
permuted to match on the host.

Device pipeline per 128-row query tile:
    outer product (DVE + GpSimd split, fp32)
    -> exp with (ln240 - rowmax) bias (ACT, fp8 out)
    -> XBAR DMA-transpose pieces of the fp16-viewed E (SP)
    -> 32 accumulated fp8x fp16 matmuls vs [V | 1] chunks (PE); column
       256 accumulates the softmax denominator Z
    -> 1/Z normalize (DVE, fp16 out) -> batched stores (SP)

All DMA rides the SP queue (HWDGE): Tile serializes XBAR transposes
against SWDGE (gpsimd) DMAs, so SWDGE is unusable mid-kernel.  The input
loads are interleaved into SP's idle fill window ahead of the first
transposes.
"""

import sys

sys.path.insert(0, "/opt/trn_rl_repo")

import numpy as np

B, S, D = 2, 4096, 256
PD = 64  # proj_dim_l == proj_dim_r == 64, PD*PD == S
NQ = S // 4  # query rows per core
QT = NQ // 128  # query tiles per core (8)
MC = S // 128  # m-chunks (32)
PC = MC // 2  # pair-chunks (16) of 256 m each
N_CORES = 8
DZ = D + 1  # V columns + ones column for Z
ESCALE = 240.0  # fp8 softmax numerator scale (cancels in normalization)

# per-tile plans: outer pieces (engine, p0, p1), exp pieces (p0, p1) in
# p units (64 cols each), xbar/mm pieces (c0, c1) in pair-chunk units
# (256 E cols each). exp piece (p0, p1) covers pair-chunks (p0//4, p1//4).
_PLANS = {
    "fill": {  # tile 0: small leading pieces to start ACT/SP early
        "outer": [("v", 0, 8), ("v", 8, 16), ("v", 16, 28), ("p", 28, 64)],
        "exp": [(0, 8), (8, 16), (16, 28), (28, 64)],
        "xbar": [(0, 2), (2, 4), (4, 7), (7, 16)],
    },
    "mid": {
        "outer": [("v", 0, 28), ("p", 28, 64)],
        "exp": [(0, 64)],
        "xbar": [(0, 4), (4, 8), (8, 12), (12, 16)],
    },
    "tail": {  # tile 7: small trailing pieces to shorten the drain
        "outer": [("v", 0, 28), ("p", 28, 64)],
        "exp": [(0, 32), (32, 56), (56, 64)],
        "xbar": [(0, 8), (8, 14), (14, 16)],
    },
}

_CACHE = {}


def _build():
    if "nc" in _CACHE:
        return _CACHE["nc"]

    import concourse.bass as bass
    import concourse.bacc as bacc
    import concourse.tile as tile
    from concourse import mybir

    F32 = mybir.dt.float32
    F16 = mybir.dt.float16
    F8 = mybir.dt.float8e4
    EXP = mybir.ActivationFunctionType.Exp

    nc = bacc.Bacc("TRN2", target_bir_lowering=False, debug=False)

    lr_d = nc.dram_tensor("lr", [128, QT, 128], F32, kind="ExternalInput").ap()
    vh_d = nc.dram_tensor("Vh", [128, MC, DZ], F16, kind="ExternalInput").ap()
    nmx_d = nc.dram_tensor("negmx", [128, QT], F32, kind="ExternalInput").ap()
    out_d = nc.dram_tensor("out", [NQ, D], F16, kind="ExternalOutput").ap()

    def plan(t):
        return _PLANS["fill" if t == 0 else ("tail" if t == QT - 1 else "mid")]

    with tile.TileContext(nc) as tc:
        import contextlib

        with contextlib.ExitStack() as ctx:
            persist = ctx.enter_context(tc.tile_pool(name="persist", bufs=1))
            prodp = ctx.enter_context(tc.tile_pool(name="prodp", bufs=3))
            ep = ctx.enter_context(tc.tile_pool(name="ep", bufs=3))
            etp = ctx.enter_context(tc.tile_pool(name="etp", bufs=6))
            psO = ctx.enter_context(tc.tile_pool(name="psO", bufs=5, space="PSUM"))
            psW = ctx.enter_context(tc.tile_pool(name="psW", bufs=1, space="PSUM"))

            vall = persist.tile([128, MC, DZ], F16, tag="vall")
            lrsb = persist.tile([128, QT, 128], F32, tag="lrsb")
            nmx = persist.tile([128, QT], F32, tag="nmx")
            zinv = persist.tile([128, QT], F32, tag="zinv")
            osb = persist.tile([128, QT, D], F16, tag="osb")
            pw = persist.tile([128, 128], F16, tag="pw")
            scr = persist.tile([128, 1], F32, tag="scr")

            prod_t = {}
            E_t = {}
            et_t = {}
            ops_t = {}

            def outer(t):
                prod = prodp.tile([128, PD, PD], F32, tag="prod", name=f"prod{t}")
                prod_t[t] = prod
                l_ap = lrsb[:, t, 0:PD]
                r_ap = lrsb[:, t, PD : 2 * PD]
                for eng, p0, p1 in plan(t)["outer"]:
                    np_ = p1 - p0
                    l_b = l_ap[:, p0:p1].broadcast_to([128, np_, PD])
                    r_b = bass.AP(
                        tensor=r_ap.tensor,
                        offset=r_ap.offset,
                        ap=[r_ap.ap[0], [0, np_], r_ap.ap[1]],
                    )
                    if eng == "v":
                        nc.vector.tensor_mul(prod[:, p0:p1, :], l_b, r_b)
                    else:
                        nc.gpsimd.tensor_mul(prod[:, p0:p1, :], l_b, r_b)

            def expf(t):
                E = ep.tile([128, S], F8, tag="E", name=f"E{t}")
                E_t[t] = E
                pflat = prod_t[t][:].rearrange("p a b -> p (a b)")
                for p0, p1 in plan(t)["exp"]:
                    nc.scalar.activation(
                        out=E[:, p0 * PD : p1 * PD],
                        in_=pflat[:, p0 * PD : p1 * PD],
                        func=EXP,
                        bias=nmx[:, t : t + 1],
                        scale=1.0,
                    )

            def xbar(t, i):
                c0, c1 = plan(t)["xbar"][i]
                # transpose the fp16-viewed pair columns [c0*128, c1*128)
                et = etp.tile(
                    [128, c1 - c0, 128], F16, tag=f"et{c1 - c0}", name=f"et{t}_{i}"
                )
                e16 = E_t[t][:].bitcast(F16)
                nc.sync.dma_start(
                    out=et[:], in_=e16[:, c0 * 128 : c1 * 128], transpose=True
                )
                et_t[(t, i)] = et

            def mm(t, i):
                c0, c1 = plan(t)["xbar"][i]
                if i == 0:
                    ops_t[t] = psO.tile([128, DZ], F32, tag="pso", name=f"ops{t}")
                ops = ops_t[t]
                et8 = et_t[(t, i)][:].bitcast(F8)  # [128, c1-c0, 256]
                for c in range(c0, c1):
                    base = et8[:, c - c0, :]
                    for u in range(2):
                        # weights [K=128, n=128]: W[p, n] = E'[n, 256c+2p+u]
                        lhsT = bass.AP(
                            tensor=base.tensor,
                            offset=base.offset + u,
                            ap=[base.ap[0], [2, 128]],
                        )
                        cc = 2 * c + u
                        nc.tensor.matmul(
                            ops[:],
                            lhsT,
                            vall[:, cc, :],
                            start=(cc == 0),
                            stop=(cc == MC - 1),
                        )

            def epi(t):
                ops = ops_t[t]
                nc.vector.reciprocal(zinv[:, t : t + 1], ops[:, D : D + 1])
                nc.vector.tensor_scalar_mul(
                    osb[:, t, :], ops[:, 0:D], zinv[:, t : t + 1]
                )

            def store(t0, t1):
                dst = out_d[t0 * 128 : t1 * 128, :].rearrange(
                    "(t p) d -> p t d", p=128
                )
                nc.sync.dma_start(out=dst, in_=osb[:, t0:t1, :])

            # ---- emission ----
            # ACT: hoist the activation-table load behind a dummy exp.
            nc.vector.memset(pw[:], 0.03125)
            nc.scalar.activation(out=scr[:], in_=pw[:, 0:1], func=EXP)

            # SP carries all DMA: loads fill its idle window, then xbars.
            nc.sync.dma_start(out=lrsb[:, 0:2, :], in_=lr_d[:, 0:2, :])
            nc.sync.dma_start(out=nmx[:], in_=nmx_d[:])
            nc.sync.dma_start(out=lrsb[:, 2:QT, :], in_=lr_d[:, 2:QT, :])

            def vq(q):
                nc.sync.dma_start(
                    out=vall[:, q * 8 : (q + 1) * 8, :],
                    in_=vh_d[:, q * 8 : (q + 1) * 8, :],
                )

            # PE prewarm while the front fills
            pwps = psW.tile([128, 128], F32, tag="psw")
            for _ in range(24):
                nc.tensor.matmul(pwps[:], pw[:], pw[:], start=True, stop=True)

            outer(0)
            expf(0)
            vq(0)
            xbar(0, 0)
            outer(1)
            expf(1)
            vq(1)
            xbar(0, 1)
            xbar(0, 2)
            outer(2)
            xbar(0, 3)
            expf(2)
            mm(0, 0)
            mm(0, 1)
            mm(0, 2)
            xbar(1, 0)
            vq(2)
            xbar(1, 1)
            vq(3)
            outer(3)
            mm(0, 3)
            mm(1, 0)
            mm(1, 1)
            xbar(1, 2)
            xbar(1, 3)
            expf(3)
            outer(4)
            mm(1, 2)
            mm(1, 3)
            for q in range(4):
                xbar(2, q)
                mm(2, q)
            expf(4)
            outer(5)
            epi(0)
            for q in range(4):
                xbar(3, q)
                mm(3, q)
            expf(5)
            outer(6)
            epi(1)
            for q in range(4):
                xbar(4, q)
                mm(4, q)
            expf(6)
            outer(7)
            epi(2)
            for q in range(4):
                xbar(5, q)
                mm(5, q)
            expf(7)
            epi(3)
            store(0, 3)
            for q in range(4):
                xbar(6, q)
                mm(6, q)
            epi(4)
            for i in range(3):
                xbar(7, i)
                mm(7, i)
            epi(5)
            store(3, 6)
            epi(6)
            epi(7)
            store(6, 8)

    nc.compile()
    _CACHE["nc"] = nc
    return nc


def _in_maps(x, Wl, Wr, Wv, Wo):
    x = np.ascontiguousarray(x, np.float32)
    Wl = np.asarray(Wl, np.float32)
    Wr = np.asarray(Wr, np.float32)
    Wv = np.asarray(Wv, np.float32)

    import ml_dtypes

    F8NP = ml_dtypes.float8_e4m3fn

    maps = []
    for b in range(B):
        l = x[b] @ Wl  # (S, 64)
        r = x[b] @ Wr  # (S, 64)
        V = x[b] @ Wv  # (S, 256) fp32
        # device m order: chunk cc=2c+u, partition p <-> m = 256c + 2p + u
        m = np.arange(S)
        c, rem = m // 256, m % 256
        perm = np.empty(S, np.int64)  # perm[cc*128 + p] = m
        cc = 2 * c + (rem % 2)
        p = rem // 2
        perm[cc * 128 + p] = m
        Vp = (V[perm]).astype(np.float16)  # (S, 256) in device order
        Vh = np.empty((128, MC, DZ), np.float16)
        Vh[:, :, 0:D] = Vp.reshape(MC, 128, D).transpose(1, 0, 2)
        Vh[:, :, D] = np.float16(1.0)
        # exact per-row max of the outer product via corners
        lmax, lmin = l.max(1), l.min(1)
        rmax, rmin = r.max(1), r.min(1)
        mx = np.maximum.reduce([lmax * rmax, lmax * rmin, lmin * rmax, lmin * rmin])
        bias = np.float32(np.log(ESCALE)) - mx  # exp(attn + bias) <= 240
        lr = np.concatenate([l, r], axis=1)  # (S, 128)
        for cq in range(4):
            q = cq * NQ
            lrc = lr[q : q + NQ].reshape(QT, 128, 128).transpose(1, 0, 2)
            nmxc = bias[q : q + NQ].reshape(QT, 128).T
            maps.append(
                {
                    "lr": np.ascontiguousarray(lrc),
                    "Vh": Vh,
                    "negmx": np.ascontiguousarray(nmxc, np.float32),
                }
            )
    return maps


def kernel(x, Wl, Wr, Wv, Wo, _trace=False, _result_holder=None):
    from concourse.bass_utils import run_bass_kernel_spmd

    nc = _build()
    maps = _in_maps(x, Wl, Wr, Wv, Wo)
    res = run_bass_kernel_spmd(nc, maps, list(range(N_CORES)), trace=_trace)
    if _result_holder is not None:
        _result_holder.append(res)
    Wo32 = np.asarray(Wo, np.float32)
    out = np.empty((B, S, D), np.float32)
    for c in range(N_CORES):
        b, q = c // 4, (c % 4) * NQ
        out[b, q : q + NQ] = res.results[c]["out"].astype(np.float32) @ Wo32
    return out


# revision 24
# speedup vs baseline: 1.0338x; 1.0338x over previous
"""Multi-head factorized dense attention on 8 TRN2 NeuronCores.

Reference computation (per batch b):
    V = x @ Wv                      (4096, 256)
    l = x @ Wl, r = x @ Wr          (4096, 64) each
    attn[n, p*64+q] = l[n,p]*r[n,q] (4096, 4096)
    score = softmax(attn, -1)
    o = score @ V                   (shared across heads == plain matmul)
    out = o @ Wo

Sharding: 8 cores = 2 batches x 4 query-row chunks of 1024 rows.

Host precomputes the small dense projections (l, r, V with an appended
ones-column, exact per-row score max via the outer-product corner trick)
and applies Wo to the gathered device output.  The device does only the
O(S^2) part: outer product -> exp -> transpose -> score @ [V | 1].

E is stored as fp8e4m3 scaled by 240 (folded into the exp bias; the
scale cancels in the softmax normalization).  The fp8 E is transposed
through the XBAR viewed as fp16 pairs — half the transpose cost — which
lands m=256c+2p+u at partition p of pair-chunk c.  Each pair-chunk is
contracted with two stride-2 fp8 weight matmuls against fp16 V rows
permuted to match on the host.

Device pipeline per 128-row query tile:
    outer product (DVE + GpSimd split, fp32)
    -> exp with (ln240 - rowmax) bias (ACT, fp8 out)
    -> XBAR DMA-transpose pieces of the fp16-viewed E (SP)
    -> 32 accumulated fp8x fp16 matmuls vs [V | 1] chunks (PE); column
       256 accumulates the softmax denominator Z
    -> 1/Z normalize (DVE, fp16 out) -> batched stores (SP)

All DMA rides the SP queue (HWDGE): Tile serializes XBAR transposes
against SWDGE (gpsimd) DMAs, so SWDGE is unusable mid-kernel.  The input
loads are interleaved into SP's idle fill window ahead of the first
transposes.
"""

import sys

sys.path.insert(0, "/opt/trn_rl_repo")

import numpy as np

B, S, D = 2, 4096, 256
PD = 64  # proj_dim_l == proj_dim_r == 64, PD*PD == S
NQ = S // 4  # query rows per core
QT = NQ // 128  # query tiles per core (8)
MC = S // 128  # m-chunks (32)
PC = MC // 2  # pair-chunks (16) of 256 m each
N_CORES = 8
DZ = D + 1  # V columns + ones column for Z
ESCALE = 240.0  # fp8 softmax numerator scale (cancels in normalization)

# per-tile plans: outer pieces (engine, p0, p1), exp pieces (p0, p1) in
# p units (64 cols each), xbar/mm pieces (c0, c1) in pair-chunk units
# (256 E cols each). exp piece (p0, p1) covers pair-chunks (p0//4, p1//4).
_PLANS = {
    "fill": {  # tile 0: small leading pieces to start ACT/SP early
        "outer": [("v", 0, 8), ("v", 8, 20), ("v", 20, 28), ("p", 28, 64)],
        "exp": [(0, 8), (8, 20), (20, 28), (28, 64)],
        "xbar": [(0, 2), (2, 5), (5, 7), (7, 16)],
    },
    "mid": {
        "outer": [("v", 0, 28), ("p", 28, 64)],
        "exp": [(0, 64)],
        "xbar": [(0, 4), (4, 8), (8, 12), (12, 16)],
    },
    "tail": {  # tile 7: small trailing pieces to shorten the drain
        "outer": [("v", 0, 28), ("p", 28, 64)],
        "exp": [(0, 32), (32, 56), (56, 64)],
        "xbar": [(0, 8), (8, 14), (14, 16)],
    },
}

_CACHE = {}


def _build():
    if "nc" in _CACHE:
        return _CACHE["nc"]

    import concourse.bass as bass
    import concourse.bacc as bacc
    import concourse.tile as tile
    from concourse import mybir

    F32 = mybir.dt.float32
    F16 = mybir.dt.float16
    F8 = mybir.dt.float8e4
    EXP = mybir.ActivationFunctionType.Exp

    nc = bacc.Bacc("TRN2", target_bir_lowering=False, debug=False)

    lr_d = nc.dram_tensor("lr", [128, QT, 128], F32, kind="ExternalInput").ap()
    vh_d = nc.dram_tensor("Vh", [128, MC, DZ], F16, kind="ExternalInput").ap()
    nmx_d = nc.dram_tensor("negmx", [128, QT], F32, kind="ExternalInput").ap()
    out_d = nc.dram_tensor("out", [NQ, D], F16, kind="ExternalOutput").ap()

    def plan(t):
        return _PLANS["fill" if t == 0 else ("tail" if t == QT - 1 else "mid")]

    with tile.TileContext(nc) as tc:
        import contextlib

        with contextlib.ExitStack() as ctx:
            persist = ctx.enter_context(tc.tile_pool(name="persist", bufs=1))
            prodp = ctx.enter_context(tc.tile_pool(name="prodp", bufs=3))
            ep = ctx.enter_context(tc.tile_pool(name="ep", bufs=3))
            etp = ctx.enter_context(tc.tile_pool(name="etp", bufs=6))
            psO = ctx.enter_context(tc.tile_pool(name="psO", bufs=5, space="PSUM"))
            psW = ctx.enter_context(tc.tile_pool(name="psW", bufs=1, space="PSUM"))

            vall = persist.tile([128, MC, DZ], F16, tag="vall")
            lrsb = persist.tile([128, QT, 128], F32, tag="lrsb")
            nmx = persist.tile([128, QT], F32, tag="nmx")
            zinv = persist.tile([128, QT], F32, tag="zinv")
            osb = persist.tile([128, QT, D], F16, tag="osb")
            pw = persist.tile([128, 128], F16, tag="pw")
            scr = persist.tile([128, 1], F32, tag="scr")

            prod_t = {}
            E_t = {}
            et_t = {}
            ops_t = {}

            def outer(t):
                prod = prodp.tile([128, PD, PD], F32, tag="prod", name=f"prod{t}")
                prod_t[t] = prod
                l_ap = lrsb[:, t, 0:PD]
                r_ap = lrsb[:, t, PD : 2 * PD]
                for eng, p0, p1 in plan(t)["outer"]:
                    np_ = p1 - p0
                    l_b = l_ap[:, p0:p1].broadcast_to([128, np_, PD])
                    r_b = bass.AP(
                        tensor=r_ap.tensor,
                        offset=r_ap.offset,
                        ap=[r_ap.ap[0], [0, np_], r_ap.ap[1]],
                    )
                    if eng == "v":
                        nc.vector.tensor_mul(prod[:, p0:p1, :], l_b, r_b)
                    else:
                        nc.gpsimd.tensor_mul(prod[:, p0:p1, :], l_b, r_b)

            def expf(t):
                E = ep.tile([128, S], F8, tag="E", name=f"E{t}")
                E_t[t] = E
                pflat = prod_t[t][:].rearrange("p a b -> p (a b)")
                for p0, p1 in plan(t)["exp"]:
                    nc.scalar.activation(
                        out=E[:, p0 * PD : p1 * PD],
                        in_=pflat[:, p0 * PD : p1 * PD],
                        func=EXP,
                        bias=nmx[:, t : t + 1],
                        scale=1.0,
                    )

            def xbar(t, i):
                c0, c1 = plan(t)["xbar"][i]
                # transpose the fp16-viewed pair columns [c0*128, c1*128)
                et = etp.tile(
                    [128, c1 - c0, 128], F16, tag=f"et{c1 - c0}", name=f"et{t}_{i}"
                )
                e16 = E_t[t][:].bitcast(F16)
                nc.sync.dma_start(
                    out=et[:], in_=e16[:, c0 * 128 : c1 * 128], transpose=True
                )
                et_t[(t, i)] = et

            def mm(t, i):
                c0, c1 = plan(t)["xbar"][i]
                if i == 0:
                    ops_t[t] = psO.tile([128, DZ], F32, tag="pso", name=f"ops{t}")
                ops = ops_t[t]
                et8 = et_t[(t, i)][:].bitcast(F8)  # [128, c1-c0, 256]
                for c in range(c0, c1):
                    base = et8[:, c - c0, :]
                    for u in range(2):
                        # weights [K=128, n=128]: W[p, n] = E'[n, 256c+2p+u]
                        lhsT = bass.AP(
                            tensor=base.tensor,
                            offset=base.offset + u,
                            ap=[base.ap[0], [2, 128]],
                        )
                        cc = 2 * c + u
                        nc.tensor.matmul(
                            ops[:],
                            lhsT,
                            vall[:, cc, :],
                            start=(cc == 0),
                            stop=(cc == MC - 1),
                        )

            def epi(t):
                ops = ops_t[t]
                nc.vector.reciprocal(zinv[:, t : t + 1], ops[:, D : D + 1])
                nc.vector.tensor_scalar_mul(
                    osb[:, t, :], ops[:, 0:D], zinv[:, t : t + 1]
                )

            def store(t0, t1):
                dst = out_d[t0 * 128 : t1 * 128, :].rearrange(
                    "(t p) d -> p t d", p=128
                )
                nc.sync.dma_start(out=dst, in_=osb[:, t0:t1, :])

            # ---- emission ----
            # ACT: hoist the activation-table load behind a dummy exp.
            nc.vector.memset(pw[:], 0.03125)
            nc.scalar.activation(out=scr[:], in_=pw[:, 0:1], func=EXP)

            # SP carries all DMA: loads fill its idle window, then xbars.
            nc.sync.dma_start(out=lrsb[:, 0:2, :], in_=lr_d[:, 0:2, :])
            nc.sync.dma_start(out=nmx[:], in_=nmx_d[:])
            nc.sync.dma_start(out=lrsb[:, 2:QT, :], in_=lr_d[:, 2:QT, :])

            def vq(q):
                nc.sync.dma_start(
                    out=vall[:, q * 8 : (q + 1) * 8, :],
                    in_=vh_d[:, q * 8 : (q + 1) * 8, :],
                )

            # PE prewarm while the front fills
            pwps = psW.tile([128, 128], F32, tag="psw")
            for _ in range(24):
                nc.tensor.matmul(pwps[:], pw[:], pw[:], start=True, stop=True)

            outer(0)
            expf(0)
            vq(0)
            xbar(0, 0)
            outer(1)
            expf(1)
            vq(1)
            xbar(0, 1)
            xbar(0, 2)
            outer(2)
            xbar(0, 3)
            expf(2)
            mm(0, 0)
            mm(0, 1)
            mm(0, 2)
            xbar(1, 0)
            vq(2)
            xbar(1, 1)
            vq(3)
            outer(3)
            mm(0, 3)
            mm(1, 0)
            mm(1, 1)
            xbar(1, 2)
            xbar(1, 3)
            expf(3)
            outer(4)
            mm(1, 2)
            mm(1, 3)
            for q in range(4):
                xbar(2, q)
                mm(2, q)
            expf(4)
            outer(5)
            epi(0)
            for q in range(4):
                xbar(3, q)
                mm(3, q)
            expf(5)
            outer(6)
            epi(1)
            for q in range(4):
                xbar(4, q)
                mm(4, q)
            expf(6)
            outer(7)
            epi(2)
            for q in range(4):
                xbar(5, q)
                mm(5, q)
            expf(7)
            epi(3)
            store(0, 3)
            for q in range(4):
                xbar(6, q)
                mm(6, q)
            epi(4)
            for i in range(3):
                xbar(7, i)
                mm(7, i)
            epi(5)
            store(3, 6)
            epi(6)
            epi(7)
            store(6, 8)

    nc.compile()
    _CACHE["nc"] = nc
    return nc


def _in_maps(x, Wl, Wr, Wv, Wo):
    x = np.ascontiguousarray(x, np.float32)
    Wl = np.asarray(Wl, np.float32)
    Wr = np.asarray(Wr, np.float32)
    Wv = np.asarray(Wv, np.float32)

    import ml_dtypes

    F8NP = ml_dtypes.float8_e4m3fn

    maps = []
    for b in range(B):
        l = x[b] @ Wl  # (S, 64)
        r = x[b] @ Wr  # (S, 64)
        V = x[b] @ Wv  # (S, 256) fp32
        # device m order: chunk cc=2c+u, partition p <-> m = 256c + 2p + u
        m = np.arange(S)
        c, rem = m // 256, m % 256
        perm = np.empty(S, np.int64)  # perm[cc*128 + p] = m
        cc = 2 * c + (rem % 2)
        p = rem // 2
        perm[cc * 128 + p] = m
        Vp = (V[perm]).astype(np.float16)  # (S, 256) in device order
        Vh = np.empty((128, MC, DZ), np.float16)
        Vh[:, :, 0:D] = Vp.reshape(MC, 128, D).transpose(1, 0, 2)
        Vh[:, :, D] = np.float16(1.0)
        # exact per-row max of the outer product via corners
        lmax, lmin = l.max(1), l.min(1)
        rmax, rmin = r.max(1), r.min(1)
        mx = np.maximum.reduce([lmax * rmax, lmax * rmin, lmin * rmax, lmin * rmin])
        bias = np.float32(np.log(ESCALE)) - mx  # exp(attn + bias) <= 240
        lr = np.concatenate([l, r], axis=1)  # (S, 128)
        for cq in range(4):
            q = cq * NQ
            lrc = lr[q : q + NQ].reshape(QT, 128, 128).transpose(1, 0, 2)
            nmxc = bias[q : q + NQ].reshape(QT, 128).T
            maps.append(
                {
                    "lr": np.ascontiguousarray(lrc),
                    "Vh": Vh,
                    "negmx": np.ascontiguousarray(nmxc, np.float32),
                }
            )
    return maps


def kernel(x, Wl, Wr, Wv, Wo, _trace=False, _result_holder=None):
    from concourse.bass_utils import run_bass_kernel_spmd

    nc = _build()
    maps = _in_maps(x, Wl, Wr, Wv, Wo)
    res = run_bass_kernel_spmd(nc, maps, list(range(N_CORES)), trace=_trace)
    if _result_holder is not None:
        _result_holder.append(res)
    Wo32 = np.asarray(Wo, np.float32)
    out = np.empty((B, S, D), np.float32)
    for c in range(N_CORES):
        b, q = c // 4, (c % 4) * NQ
        out[b, q : q + NQ] = res.results[c]["out"].astype(np.float32) @ Wo32
    return out
